# revision 20
# baseline (speedup 1.0000x reference)
"""Trainium2 Bass kernel for nn_CausalNet (block-diagonal GNN + BN + MLP head).

Strategy: data-parallel over batch (8 samples/core on 8 cores).
 - Feature-major layouts throughout so every BN/broadcast is per-partition.
 - Partition-axis reductions (row norms, degrees) via ones-column matmuls on
   TensorE instead of gpsimd tensor_reduce.
 - Heavy matmul paths (gram, XW, A*XW, readout) run in bf16 at full PE rate;
   accumulation stays fp32 in PSUM. BN statistics and affines stay fp32.
 - Both D^-1/2 scales folded into the stationary A^T via rank-1 outer matmuls.
 - BatchNorm stats cross-core via 2KB AllReduce x2.
 - Readout [64,131072]@[131072,128]: AllToAll (split into two feature halves,
   pipelined with the BN2 apply) redistributes h2 so each core contracts only
   its 16384-row slice of Wm1 (bf16, 4.2MB streamed). Receive-side loads land
   contiguously in sample-major tiles; PE transposes produce the rhs tiles
   on-chip (no strided gather DMA). A 32KB AllReduce combines the partials.
"""
import sys
import numpy as np

sys.path.insert(0, "/opt/trn_rl_repo")

B, N, P, D = 64, 4, 128, 256
H = 256
TOTP = N * P          # 512
NCORES = 8
BLOC = B // NCORES    # 8 samples per core
T = BLOC * TOTP       # 4096 tokens per core
NB = BLOC * N         # 32 (sample, subgraph) blocks per core
FEAT = TOTP * H       # 131072
FSL = FEAT // NCORES  # 16384 Wm1 rows per core
TSL = TOTP // NCORES  # 64 patches per core slice
JT = H // 128         # 2 feature partition-tiles
NCH = 64              # readout chunks per half
WMG = 8               # chunks per streaming group
EPS_BN = 1e-5
CNT1 = float(B * TOTP)   # BN denominator for GCN layers
CNT2 = float(B)          # BN denominator for head


def build_bass(repeat=1, no_cc=False):
    import concourse.bass as bass
    import concourse.bacc as bacc
    import concourse.mybir as mybir
    import concourse.tile as tile

    f32 = mybir.dt.float32
    b16 = mybir.dt.bfloat16
    Act = mybir.ActivationFunctionType
    Alu = mybir.AluOpType
    AX = mybir.AxisListType

    nc = bacc.Bacc("TRN2", target_bir_lowering=False, debug=False,
                   num_devices=NCORES)

    def inp(name, shape, dt=None):
        return nc.dram_tensor(name, shape, dt or f32, kind="ExternalInput")

    xT_d = inp("xT", [D, T])            # d-major activations for this core
    W1_d = inp("W1", [D, H])
    b1_d = inp("b1r", [1, H])
    g1_d = inp("g1p", [128, JT])        # column jh = features [jh*128,(jh+1)*128)
    be1_d = inp("be1p", [128, JT])
    W2_d = inp("W2", [H, H])
    b2_d = inp("b2r", [1, H])
    g2_d = inp("g2p", [128, JT])
    be2_d = inp("be2p", [128, JT])
    mAT_d = inp("mAT", [P, 4 * P])      # (0.5*mask*(1-I)).T tiled x4
    mBT_d = inp("mBT", [P, 4 * P])      # (0.5*mask*(1-I) + I).T tiled x4
    Wm1_d = inp("Wm1s", [FSL, 128], b16)  # this core's Wm1 slice, flat-major
    gm1_d = inp("gm1", [128, 1])
    bem1_d = inp("bem1", [128, 1])
    Wm2_d = inp("Wm2", [128, 64])
    gm2_d = inp("gm2", [64, 1])
    bem2_d = inp("bem2", [64, 1])
    Wm3_d = inp("Wm3", [64, 2])
    bm3_d = inp("bm3", [2, 1])
    onec_d = inp("ones_col", [128, 1])
    oner_d = inp("ones_row", [1, 128])
    id_d = inp("ident", [128, 128])

    out_ext = nc.dram_tensor("out", [2, B], f32, kind="ExternalOutput")

    with tile.TileContext(nc) as tc:
        with (
            tc.tile_pool(name="persist", bufs=1) as pp,
            tc.tile_pool(name="work", bufs=3) as wp,
            tc.tile_pool(name="small", bufs=2) as sp,
            tc.tile_pool(name="scratch", bufs=2) as scp,
            tc.tile_pool(name="wm1", bufs=4) as wmp,
            tc.tile_pool(name="ps", bufs=2, space="PSUM") as ps,
            tc.tile_pool(name="dram", bufs=1, space="DRAM") as dp,
        ):
            # ---------------- persistent SBUF ----------------
            def load(name, dram, shape, sl=None):
                t = pp.tile(shape, f32, tag=name, name=name)
                nc.gpsimd.dma_start(out=t[:], in_=dram[:] if sl is None else sl)
                return t

            xTs = [load(f"xT{k}", xT_d, [128, T], xT_d[k * 128:(k + 1) * 128, :])
                   for k in range(2)]
            W1s = [load(f"W1{k}", W1_d, [128, H], W1_d[k * 128:(k + 1) * 128, :])
                   for k in range(2)]
            W2s = [load(f"W2{k}", W2_d, [128, H], W2_d[k * 128:(k + 1) * 128, :])
                   for k in range(2)]
            b1s = load("b1", b1_d, [1, H])
            b2s = load("b2", b2_d, [1, H])
            mATs = load("mAT", mAT_d, [P, 4 * P])
            mBTs = load("mBT", mBT_d, [P, 4 * P])
            onec = load("onec", onec_d, [128, 1])
            oner = load("oner", oner_d, [1, 128])
            idents = load("ident", id_d, [128, 128])
            g1s = load("g1", g1_d, [128, JT])
            be1s = load("be1", be1_d, [128, JT])
            g2s = load("g2", g2_d, [128, JT])
            be2s = load("be2", be2_d, [128, JT])
            gm1s = load("gm1", gm1_d, [128, 1])
            bem1s = load("bem1", bem1_d, [128, 1])
            gm2s = load("gm2", gm2_d, [64, 1])
            bem2s = load("bem2", bem2_d, [64, 1])
            Wm2s = load("Wm2", Wm2_d, [128, 64])
            Wm3s = load("Wm3", Wm3_d, [64, 2])
            bm3s = load("bm3", bm3_d, [2, 1])

            # bf16 casts of matmul operands
            def cast16(name, src, shape):
                t = pp.tile(shape, b16, tag=name, name=name)
                nc.vector.tensor_copy(t[:], src)
                return t

            xB = [cast16(f"xB{k}", xTs[k][:], [128, T]) for k in range(2)]
            W1b = [cast16(f"W1b{k}", W1s[k][:], [128, H]) for k in range(2)]
            W2b = [cast16(f"W2b{k}", W2s[k][:], [128, H]) for k in range(2)]
            b1b = cast16("b1b", b1s[:], [1, H])
            b2b = cast16("b2b", b2s[:], [1, H])
            onecb = cast16("onecb", onec[:], [128, 1])
            onerb = cast16("onerb", oner[:], [1, 128])
            identb = cast16("identb", idents[:], [128, 128])

            epsb = pp.tile([128, 1], f32, tag="epsb")
            nc.vector.memset(epsb[:], EPS_BN)
            rinv = pp.tile([1, T], f32, tag="rinv")          # 1/||x_t||
            AnT = pp.tile([128, NB * P], b16, tag="AnT")     # scaled A^T blocks
            h1T = [pp.tile([128, T], b16, tag=f"h1T{k}", name=f"h1T{k}") for k in range(JT)]
            h2T = [pp.tile([128, T], b16, tag=f"h2T{k}", name=f"h2T{k}") for k in range(JT)]

            rg = [list(range(NCORES))]

            def cc(kind, op, cin, cout):
                if no_cc:
                    nc.sync.dma_start(out=cout[:], in_=cin[:])
                else:
                    nc.gpsimd.collective_compute(
                        kind, op, replica_groups=rg,
                        ins=[cin.opt()], outs=[cout.opt()])

            for _rep in range(repeat):
                st1_in = dp.tile([128, 4], f32, tag="st1i", name="st1_in")
                st1_out = dp.tile([128, 4], f32, tag="st1o", addr_space="Shared", name="st1_out")
                st2_in = dp.tile([128, 4], f32, tag="st2i", name="st2_in")
                st2_out = dp.tile([128, 4], f32, tag="st2o", addr_space="Shared", name="st2_out")
                a2a_in = [dp.tile([NCORES, BLOC, 128, TSL], b16, tag=f"a2ai{j}",
                                  name=f"a2a_in{j}") for j in range(JT)]
                a2a_out = [dp.tile([NCORES, BLOC, 128, TSL], b16, tag=f"a2ao{j}",
                                   name=f"a2a_out{j}") for j in range(JT)]
                z1_io = [(dp.tile([128, 64], f32, tag=f"z1i{j}", name=f"z1_in{j}"),
                          dp.tile([128, 64], f32, tag=f"z1o{j}", addr_space="Shared",
                                  name=f"z1_out{j}")) for j in range(JT)]

                NG = JT * NCH          # 128 readout chunks total
                NGRP = NG // WMG       # 16 groups of 8 chunks
                wm1_gkm = Wm1_d[:].rearrange("(g k) m -> k g m", k=128)
                recv_tiles = [None] * NGRP
                wmb_tiles = [None] * NGRP

                def load_wmb(gi):
                    wb = wmp.tile([128, WMG * 128], b16, tag="wmb")
                    nc.sync.dma_start(
                        out=wb[:].rearrange("k (g m) -> k g m", g=WMG),
                        in_=wm1_gkm[:, gi * WMG:(gi + 1) * WMG, :])
                    wmb_tiles[gi] = wb

                def load_recv(gi):
                    # receive: [64 samples, 8 chunks * 128 flat], 2KB rows
                    jh, kg = gi // (NGRP // JT), gi % (NGRP // JT)
                    rv = wp.tile([64, WMG * 128], b16, tag="recv", bufs=4)
                    a2a_flat = a2a_out[jh][:].rearrange("r s f t -> (r s) (f t)")
                    nc.sync.dma_start(
                        out=rv[:],
                        in_=a2a_flat[:, kg * WMG * 128:(kg + 1) * WMG * 128])
                    recv_tiles[gi] = rv

                # Wm1 stream starts immediately; DMA is idle during the GCN phase
                for gi in range(3):
                    load_wmb(gi)

                # ---- row norms: rinv[t] = 1/||x_t|| via ones-matmul ----
                for c8 in range(8):
                    cl = c8 * 512
                    red = ps.tile([1, 512], f32, tag="hh", bufs=2)
                    for kt in range(2):
                        sq = scp.tile([128, 512], b16, tag="sq", bufs=2)
                        nc.scalar.activation(sq[:], xB[kt][:, cl:cl + 512],
                                             Act.Square)
                        nc.tensor.matmul(red[:], onecb[:], sq[:],
                                         start=(kt == 0), stop=(kt == 1))
                    nc.scalar.activation(rinv[:, cl:cl + 512], red[:],
                                         Act.Abs_reciprocal_sqrt)

                # ------- adjacency + layer1, 4 blocks per PSUM bank -------
                SB = NB // 4
                for sb in range(SB):
                    c0 = sb * 4 * P
                    G4 = ps.tile([P, 4 * P], f32, tag="G")
                    for b in range(4):
                        cb = c0 + b * P
                        for kt in range(2):
                            nc.tensor.matmul(
                                G4[:, b * P:(b + 1) * P],
                                xB[kt][:, cb:cb + P], xB[kt][:, cb:cb + P],
                                start=(kt == 0), stop=(kt == 1),
                            )
                    R4 = ps.tile([P, 4 * P], f32, tag="adj")
                    for b in range(4):
                        cb = c0 + b * P
                        nc.tensor.matmul(R4[:, b * P:(b + 1) * P],
                                         rinv[:, cb:cb + P], rinv[:, cb:cb + P],
                                         start=True, stop=True)
                    t1 = wp.tile([P, 4 * P], f32, tag="t1")
                    nc.vector.tensor_mul(t1[:], G4[:], mATs[:])
                    AT = wp.tile([P, 4 * P], f32, tag="AT")
                    nc.vector.tensor_mul(AT[:], R4[:], t1[:])
                    nc.vector.tensor_add(AT[:], AT[:], mBTs[:])
                    # degree via ones-matmul (columns of A^T sum over partitions)
                    dg_ps = ps.tile([1, 512], f32, tag="hh", bufs=2)
                    nc.tensor.matmul(dg_ps[:], onec[:], AT[:],
                                     start=True, stop=True)
                    dinv = sp.tile([1, 4 * P], f32, tag="dinv")
                    nc.scalar.activation(dinv[:], dg_ps[:],
                                         Act.Abs_reciprocal_sqrt)
                    Do4 = ps.tile([P, 4 * P], f32, tag="adj")
                    for b in range(4):
                        nc.tensor.matmul(Do4[:, b * P:(b + 1) * P],
                                         dinv[:, b * P:(b + 1) * P],
                                         dinv[:, b * P:(b + 1) * P],
                                         start=True, stop=True)
                    nc.vector.tensor_mul(AnT[:, c0:c0 + 4 * P], AT[:], Do4[:])

                    xws = []
                    for b in range(4):
                        cb = c0 + b * P
                        xw_ps = ps.tile([128, H], f32, tag="xw")
                        for kt in range(2):
                            nc.tensor.matmul(
                                xw_ps[:], xB[kt][:, cb:cb + P], W1b[kt][:],
                                start=(kt == 0), stop=False,
                            )
                        nc.tensor.matmul(xw_ps[:], onerb[:], b1b[:],
                                         start=False, stop=True)
                        xw = wp.tile([128, H], b16, tag="xw", bufs=5)
                        if b % 2 == 0:
                            nc.scalar.activation(xw[:], xw_ps[:], Act.Copy)
                        else:
                            nc.vector.tensor_copy(xw[:], xw_ps[:])
                        xws.append(xw)

                    for jh in range(JT):
                        hh4 = ps.tile([128, 4 * P], f32, tag="hh")
                        for b in range(4):
                            cb = c0 + b * P
                            nc.tensor.matmul(
                                hh4[:, b * P:(b + 1) * P],
                                xws[b][:, jh * 128:(jh + 1) * 128],
                                AnT[:, cb:cb + P],
                                start=True, stop=True,
                            )
                        nc.vector.tensor_copy(h1T[jh][:, c0:c0 + 4 * P], hh4[:])

                # ---------------- BN stats + allreduce + apply (shared) --------
                def bn_stats(hT, stin, stout):
                    st = sp.tile([128, 4], f32, tag="st")
                    for jh in range(JT):
                        nc.vector.reduce_sum(st[:, jh:jh + 1], hT[jh][:], AX.X)
                        p8 = sp.tile([128, 8], f32, tag="p8")
                        for c8 in range(8):
                            sq = scp.tile([128, 512], b16, tag="sq", bufs=2)
                            nc.scalar.activation(sq[:], hT[jh][:, c8 * 512:(c8 + 1) * 512],
                                                 Act.Square,
                                                 accum_out=p8[:, c8:c8 + 1])
                        nc.vector.reduce_sum(st[:, 2 + jh:3 + jh], p8[:], AX.X)
                    nc.sync.dma_start(out=stin[:], in_=st[:])
                    nc.gpsimd.collective_compute(
                        "AllReduce", Alu.add, replica_groups=rg,
                        ins=[stin.opt()], outs=[stout.opt()],
                    )
                    stg = sp.tile([128, 4], f32, tag="stg")
                    nc.sync.dma_start(out=stg[:], in_=stout[:])
                    return stg

                def bn_affine(stg, jh, gs, bes):
                    mean = sp.tile([128, 1], f32, tag="mean")
                    nc.vector.tensor_scalar_mul(mean[:], stg[:, jh:jh + 1], 1.0 / CNT1)
                    msq = sp.tile([128, 1], f32, tag="msq")
                    nc.vector.tensor_mul(msq[:], mean[:], mean[:])
                    var = sp.tile([128, 1], f32, tag="var")
                    nc.vector.tensor_scalar_mul(var[:], stg[:, 2 + jh:3 + jh],
                                                1.0 / CNT1)
                    nc.vector.tensor_sub(var[:], var[:], msq[:])
                    sd = sp.tile([128, 1], f32, tag="sd")
                    nc.scalar.activation(sd[:], var[:], Act.Sqrt, bias=epsb[:var.shape[0], :])
                    rsd = sp.tile([128, 1], f32, tag="rsd")
                    nc.vector.reciprocal(rsd[:], sd[:])
                    a = sp.tile([128, 1], f32, tag="a")
                    nc.vector.tensor_mul(a[:], gs[:, jh:jh + 1], rsd[:])
                    c = sp.tile([128, 1], f32, tag="c")
                    nc.vector.tensor_mul(c[:], mean[:], a[:])
                    nc.vector.tensor_sub(c[:], bes[:, jh:jh + 1], c[:])
                    return a, c

                stg1 = bn_stats(h1T, st1_in, st1_out)
                bn1_ac = [bn_affine(stg1, jh, g1s, be1s) for jh in range(JT)]
                for c4 in range(8):
                    cl = c4 * 512
                    for jh in range(JT):
                        a, c = bn1_ac[jh]
                        nc.scalar.activation(h1T[jh][:, cl:cl + 512],
                                             h1T[jh][:, cl:cl + 512], Act.Relu,
                                             bias=c[:], scale=a[:])

                # ---------------- layer 2 (4-block batches) ----------------
                for sb in range(SB):
                    c0 = sb * 4 * P
                    xws = []
                    for b in range(4):
                        cb = c0 + b * P
                        xw_ps = ps.tile([128, H], f32, tag="xw")
                        for jh in range(JT):
                            nc.tensor.matmul(
                                xw_ps[:], h1T[jh][:, cb:cb + P], W2b[jh][:],
                                start=(jh == 0), stop=False,
                            )
                        nc.tensor.matmul(xw_ps[:], onerb[:], b2b[:],
                                         start=False, stop=True)
                        xw = wp.tile([128, H], b16, tag="xw", bufs=5)
                        if b % 2 == 0:
                            nc.scalar.activation(xw[:], xw_ps[:], Act.Copy)
                        else:
                            nc.vector.tensor_copy(xw[:], xw_ps[:])
                        xws.append(xw)
                    for jh in range(JT):
                        hh4 = ps.tile([128, 4 * P], f32, tag="hh")
                        for b in range(4):
                            cb = c0 + b * P
                            nc.tensor.matmul(
                                hh4[:, b * P:(b + 1) * P],
                                xws[b][:, jh * 128:(jh + 1) * 128],
                                AnT[:, cb:cb + P],
                                start=True, stop=True,
                            )
                        nc.vector.tensor_copy(h2T[jh][:, c0:c0 + 4 * P], hh4[:])

                # ---- BN2 + A2A, pipelined per feature half ----
                stg2 = bn_stats(h2T, st2_in, st2_out)
                for jh in range(JT):
                    a, c = bn_affine(stg2, jh, g2s, be2s)
                    nc.scalar.activation(h2T[jh][:], h2T[jh][:], Act.Relu,
                                         bias=c[:], scale=a[:])
                    # stage: a2a_in[cd, s, f, t] = h2T[jh][f, s*512 + cd*64 + t]
                    h2_sct = h2T[jh][:].rearrange("f (s c t) -> f s c t",
                                                  s=BLOC, c=NCORES)
                    for cd in range(NCORES):
                        dst = a2a_in[jh][cd].rearrange("s f t -> f s t")
                        eng = nc.sync if cd % 2 == 0 else nc.gpsimd
                        eng.dma_start(out=dst, in_=h2_sct[:, :, cd, :])
                    cc("AllToAll", Alu.bypass, a2a_in[jh], a2a_out[jh])

                # ---- readout: stream receive + Wm1 in chunk-groups of 8;
                # PE transposes [64,128]->[128,64] make the rhs tiles on-chip.
                for gi in range(2):
                    load_recv(gi)

                z1_ps_h = [ps.tile([128, 64], f32, tag="hh", name=f"z1ps{j}") for j in range(JT)]
                tps_tiles = [None] * NG

                def issue_transpose(g):
                    gi, k8 = g // WMG, g % WMG
                    tp = ps.tile([128, 64], b16, tag="xw")
                    nc.tensor.transpose(tp[:], recv_tiles[gi][:, k8 * 128:(k8 + 1) * 128],
                                        identb[:64, :64])
                    tps_tiles[g] = tp

                issue_transpose(0)
                for g in range(NG):
                    gi, k8 = g // WMG, g % WMG
                    if k8 == 0 and gi + 2 < NGRP:
                        load_recv(gi + 2)
                        if gi + 3 < NGRP:
                            load_wmb(gi + 3)
                    if g + 1 < NG:
                        issue_transpose(g + 1)
                    rhs = sp.tile([128, 64], b16, tag="rhs", bufs=3)
                    if g % 2 == 0:
                        nc.vector.tensor_copy(rhs[:], tps_tiles[g][:])
                    else:
                        nc.scalar.activation(rhs[:], tps_tiles[g][:], Act.Copy)
                    tps_tiles[g] = None
                    hf = g // NCH
                    nc.tensor.matmul(z1_ps_h[hf][:], wmb_tiles[gi][:, k8 * 128:(k8 + 1) * 128],
                                     rhs[:], start=(g % NCH == 0), stop=(g % NCH == NCH - 1))
                    if g % NCH == NCH - 1:
                        z1s = sp.tile([128, 64], f32, tag="z1s")
                        nc.vector.tensor_copy(z1s[:], z1_ps_h[hf][:])
                        nc.sync.dma_start(out=z1_io[hf][0][:], in_=z1s[:])
                        cc("AllReduce", Alu.add, z1_io[hf][0], z1_io[hf][1])

                z1t = sp.tile([128, 64], f32, tag="z1t")
                nc.sync.dma_start(out=z1t[:], in_=z1_io[0][1][:])
                z1u = sp.tile([128, 64], f32, tag="z1u")
                nc.sync.dma_start(out=z1u[:], in_=z1_io[1][1][:])
                nc.vector.tensor_add(z1t[:], z1t[:], z1u[:])

                # ---------------- head BN + relu ----------------
                def head_bn(zt, parts, gs, bes):
                    stm = sp.tile([parts, 1], f32, tag="hstm")
                    nc.vector.reduce_sum(stm[:], zt[:], AX.X)
                    mean = sp.tile([parts, 1], f32, tag="hmean")
                    nc.vector.tensor_scalar_mul(mean[:], stm[:], 1.0 / CNT2)
                    sqs2 = sp.tile([parts, 64], f32, tag="hsq")
                    sts = sp.tile([parts, 1], f32, tag="hsts")
                    nc.scalar.activation(sqs2[:], zt[:], Act.Square, accum_out=sts[:])
                    var = sp.tile([parts, 1], f32, tag="hvar")
                    nc.vector.tensor_scalar_mul(var[:], sts[:], 1.0 / CNT2)
                    msq = sp.tile([parts, 1], f32, tag="hmsq")
                    nc.vector.tensor_mul(msq[:], mean[:], mean[:])
                    nc.vector.tensor_sub(var[:], var[:], msq[:])
                    sd = sp.tile([parts, 1], f32, tag="hsd")
                    nc.scalar.activation(sd[:], var[:], Act.Sqrt, bias=epsb[:var.shape[0], :])
                    rsd = sp.tile([parts, 1], f32, tag="hrsd")
                    nc.vector.reciprocal(rsd[:], sd[:])
                    a = sp.tile([parts, 1], f32, tag="ha")
                    nc.vector.tensor_mul(a[:], gs[:], rsd[:])
                    c = sp.tile([parts, 1], f32, tag="hc")
                    nc.vector.tensor_mul(c[:], mean[:], a[:])
                    nc.vector.tensor_sub(c[:], bes[:], c[:])
                    nc.scalar.activation(zt[:], zt[:], Act.Relu, bias=c[:], scale=a[:])

                head_bn(z1t, 128, gm1s, bem1s)

                z2_ps = ps.tile([64, 64], f32, tag="adj")
                nc.tensor.matmul(z2_ps[:], Wm2s[:], z1t[:], start=True, stop=True)
                z2t = sp.tile([64, 64], f32, tag="z2t")
                nc.vector.tensor_copy(z2t[:], z2_ps[:])
                head_bn(z2t, 64, gm2s, bem2s)

                z3_ps = ps.tile([2, 64], f32, tag="adj")
                nc.tensor.matmul(z3_ps[:], Wm3s[:], z2t[:], start=True, stop=True)
                z3 = sp.tile([2, 64], f32, tag="z3")
                nc.vector.tensor_scalar_add(z3[:], z3_ps[:], bm3s[:])
                nc.sync.dma_start(out=out_ext[:], in_=z3[:])

    nc.finalize()
    return nc


_CACHE = {}


def prepare_in_maps(inputs):
    import ml_dtypes

    x = np.asarray(inputs["x"], np.float32)
    mask = np.asarray(inputs["edge_prior_mask"], np.float32)
    Wm1 = np.asarray(inputs["Wm1"], np.float32)

    mA = 0.5 * mask * (1.0 - np.eye(P, dtype=np.float32))
    mB = mA + np.eye(P, dtype=np.float32)

    def c2(v, parts):  # [2*parts] -> [parts, 2] column-per-tile packing
        return np.ascontiguousarray(
            np.asarray(v, np.float32).reshape(2, parts).T)

    common = {
        "W1": np.asarray(inputs["W1"], np.float32),
        "b1r": np.asarray(inputs["b1"], np.float32).reshape(1, H),
        "g1p": c2(inputs["g1"], 128), "be1p": c2(inputs["be1"], 128),
        "W2": np.asarray(inputs["W2"], np.float32),
        "b2r": np.asarray(inputs["b2"], np.float32).reshape(1, H),
        "g2p": c2(inputs["g2"], 128), "be2p": c2(inputs["be2"], 128),
        "mAT": np.ascontiguousarray(np.tile(mA.T, (1, 4))),
        "mBT": np.ascontiguousarray(np.tile(mB.T, (1, 4))),
        "gm1": np.asarray(inputs["gm1"], np.float32).reshape(128, 1),
        "bem1": np.asarray(inputs["bem1"], np.float32).reshape(128, 1),
        "Wm2": np.asarray(inputs["Wm2"], np.float32),
        "gm2": np.asarray(inputs["gm2"], np.float32).reshape(64, 1),
        "bem2": np.asarray(inputs["bem2"], np.float32).reshape(64, 1),
        "Wm3": np.asarray(inputs["Wm3"], np.float32),
        "bm3": np.asarray(inputs["bm3"], np.float32).reshape(2, 1),
        "ones_col": np.ones((128, 1), np.float32),
        "ones_row": np.ones((1, 128), np.float32),
        "ident": np.eye(128, dtype=np.float32),
    }
    in_maps = []
    for c in range(NCORES):
        xc = x[c * BLOC:(c + 1) * BLOC].reshape(T, D)
        m = dict(common)
        m["xT"] = np.ascontiguousarray(xc.T)
        ws = Wm1[c * FSL:(c + 1) * FSL, :].reshape(TSL, H, 128)
        m["Wm1s"] = np.ascontiguousarray(
            ws.transpose(1, 0, 2).reshape(FSL, 128)).astype(ml_dtypes.bfloat16)
        in_maps.append(m)
    return in_maps


def kernel(**inputs):
    import concourse.bass_utils as bass_utils

    in_maps = prepare_in_maps(inputs)
    if "nc" not in _CACHE:
        _CACHE["nc"] = build_bass()
    res = bass_utils.run_bass_kernel_spmd(
        _CACHE["nc"], in_maps, core_ids=list(range(NCORES)))
    _CACHE["last"] = res
    out = res.results[0]["out"]  # [2, 64]
    return np.ascontiguousarray(np.asarray(out).T)
